# revision 19
# baseline (speedup 1.0000x reference)
"""Trainium2 Bass kernel for nn_AttentiveAutoEncoder.

Key structure: the input embedding is Linear(1,E), so the token embedding
h[b,f,:] = x[b,f] * W_emb[0,:] is rank-1.  All of q/k/v and the MHA in_proj
outputs are therefore affine in the scalar x[b,f]:

    q2[b,f,:] = x[b,f]*u_q + c_q       (u_q, c_q host-precomputed [H])

so per head the attention scores collapse to

    scores[b,h,i,j] = (a_h x_i + c_h) x_j  +  (terms constant in j)

and the j-constant terms drop out of the softmax.  The context vector
collapses to ctx[b,i,head] = s[b,h,i] * u_v[head] + c_v with
s[b,h,i] = sum_j attn[b,h,i,j] x[b,j], so attention + out-proj becomes a
[*,NH] @ [NH,H] matmul.  Only the per-feature grouped MLP stacks remain as
real GEMM work.

Sharding: attention (and the attn output) is data-parallel over batch B.
The grouped MLP stacks are expert-parallel: each core owns F/8 = 8
features for ALL B tokens, so each 128x128 weight load is amortized over
N=2048 moving columns (4 matmuls of N=512) and the big [F,NL,H,H] weight
tensors are sharded 8x instead of replicated.  The only cross-core
exchange is an AllToAll of the tiny collapsed-attention tensor
s[B,NH,F] (~1 MB chip-wide).  Activations stay transposed ([H, tokens])
through the stack so no per-layer transposes are needed.
"""

import numpy as np

B, F, E, H, NH, OUT, NL = 2048, 64, 8, 256, 4, 64, 4
HD = H // NH
NCORES = 8
BL = B // NCORES          # 256 batch rows per core (attention shard)
BT = 128                  # batch tile (partition dim)
NBT = BL // BT            # 2 batch tiles per core
P = 128
FL = F // NCORES          # 8 features per core (grouped-GEMM shard)
TOK = B                   # tokens per feature in the grouped stacks
NCH = 512                 # matmul moving-dim chunk
RCH = 1024                # relu chunk (2 PSUM banks)


def _host_precompute(inp):
    """Collapse the attention block into a handful of small tensors."""
    f64 = lambda k: np.asarray(inp[k], dtype=np.float64)
    W_emb, Wq, bq = f64("W_emb"), f64("Wq"), f64("bq")
    Wk, bk, Wv, bv = f64("Wk"), f64("bk"), f64("Wv"), f64("bv")
    Win, bin_, Wo, bo = f64("Win"), f64("bin_"), f64("Wo"), f64("bo")
    Wq2, Wk2, Wv2 = np.split(Win, 3, axis=0)
    bq2, bk2, bv2 = np.split(bin_, 3)
    e = W_emb[0]
    uq = (e @ Wq) @ Wq2.T
    cq = bq @ Wq2.T + bq2
    uk = (e @ Wk) @ Wk2.T
    uv = (e @ Wv) @ Wv2.T
    cv = bv @ Wv2.T + bv2
    sc = 1.0 / np.sqrt(HD)
    ah = np.array([uq[h * HD:(h + 1) * HD] @ uk[h * HD:(h + 1) * HD]
                   for h in range(NH)]) * sc
    ch = np.array([cq[h * HD:(h + 1) * HD] @ uk[h * HD:(h + 1) * HD]
                   for h in range(NH)]) * sc
    # a[b,i,:] = sum_h s[b,h,i] * Mproj[h,:] + const_a
    Mproj = np.stack([uv[h * HD:(h + 1) * HD] @ Wo[:, h * HD:(h + 1) * HD].T
                      for h in range(NH)])          # [NH, H]
    const_a = cv @ Wo.T + bo                        # [H]
    return (ah.astype(np.float32), ch.astype(np.float32),
            Mproj.astype(np.float32), const_a.astype(np.float32))


def _build_graph():
    import concourse.bass as bass
    import concourse.mybir as mybir
    import concourse.tile as tile
    from concourse import bacc
    from concourse.masks import make_identity

    f32 = mybir.dt.float32
    bf16 = mybir.dt.bfloat16
    AF = mybir.ActivationFunctionType
    ALU = mybir.AluOpType
    AXL = mybir.AxisListType

    nc = bacc.Bacc(None)

    x_d = nc.declare_dram_parameter("xs", [BL, F], f32, isOutput=False)
    attc_d = nc.declare_dram_parameter("attc", [2 * NH], f32, isOutput=False)
    mproj_d = nc.declare_dram_parameter("mproj", [NH, H], bf16, isOutput=False)
    ca_d = nc.declare_dram_parameter("consta", [H], f32, isOutput=False)
    wenc_d = nc.declare_dram_parameter("wenc", [FL, NL, H, H], bf16,
                                       isOutput=False)
    benc_d = nc.declare_dram_parameter("benc", [FL, NL, H], f32, isOutput=False)
    wdec_d = nc.declare_dram_parameter("wdec", [FL, NL, H, H], bf16,
                                       isOutput=False)
    bdec_d = nc.declare_dram_parameter("bdec", [FL, NL, H], f32, isOutput=False)
    wout_d = nc.declare_dram_parameter("wout", [H, OUT], bf16, isOutput=False)
    bout_d = nc.declare_dram_parameter("bout", [OUT], f32, isOutput=False)
    pc_d = nc.declare_dram_parameter("out_pc", [TOK, FL, OUT], f32,
                                     isOutput=True)
    attn_d = nc.declare_dram_parameter("out_attn", [BL, NH, F, F], f32,
                                       isOutput=True)
    # AllToAll bounce buffers for s: block d (for dest core d) holds
    # s[nh, features d*FL..(d+1)*FL, local b].  After the collective,
    # s2a_out[src, nh, fl, b] = s of THIS core's features for src's batch.
    s2a_in = nc.dram_tensor("s_a2a_in", [NCORES, NH, FL, BL], bf16)
    s2a_out = nc.dram_tensor("s_a2a_out", [NCORES, NH, FL, BL], bf16)

    with tile.TileContext(nc) as tc:
        with (
            tc.tile_pool(name="singles", bufs=1) as singles,
            tc.tile_pool(name="att", bufs=2) as att,
            tc.tile_pool(name="att1", bufs=1) as att1,
            tc.tile_pool(name="attsm", bufs=2) as attsm,
            tc.tile_pool(name="stp", bufs=2) as stpool,
            tc.tile_pool(name="wpool", bufs=8) as wpool,
            tc.tile_pool(name="zpool", bufs=2) as zpool,
            tc.tile_pool(name="opool", bufs=2) as opool,
            tc.tile_pool(name="ocp", bufs=4) as ocp,
            tc.tile_pool(name="ps_g", bufs=2, space="PSUM") as ps_g,
            tc.tile_pool(name="ps_o", bufs=2, space="PSUM") as ps_o,
            tc.tile_pool(name="ps_t", bufs=2, space="PSUM") as ps_t,
        ):
            # ---- one-time setup ------------------------------------------
            attc_t = singles.tile([P, 2 * NH], f32)
            nc.sync.dma_start(out=attc_t,
                              in_=attc_d[:].partition_broadcast(P))
            x_t = singles.tile([P, NBT, F], f32)
            nc.sync.dma_start(
                out=x_t, in_=x_d[:, :].rearrange("(t p) f -> p t f", p=P))
            mproj_t = singles.tile([NH, H], bf16)
            nc.sync.dma_start(out=mproj_t, in_=mproj_d[:, :])
            ca_t = singles.tile([P, 2], f32)
            nc.sync.dma_start(out=ca_t,
                              in_=ca_d[:].rearrange("(c p) -> p c", p=P))
            wout_t = singles.tile([P, 2, OUT], bf16)
            nc.sync.dma_start(
                out=wout_t, in_=wout_d[:, :].rearrange("(c p) o -> p c o", p=P))
            bout_t = singles.tile([OUT, 1], f32)
            nc.sync.dma_start(out=bout_t,
                              in_=bout_d[:].rearrange("(o u) -> o u", u=1))
            ident = singles.tile([OUT, OUT], bf16)
            make_identity(nc, ident)

            # s for all heads, both b tiles: [128, NBT, NH, dest, FL] (bf16)
            s_t = singles.tile([P, NBT, NH, NCORES, FL], bf16)

            # prime DVE's vector clock on the setup DMAs so the 1-wait-slot
            # TensorTensor instructions below never need >1 sync wait
            prime_t = singles.tile([P, 2], f32)
            nc.vector.tensor_copy(prime_t[:, 0:1], x_t[:, 0, 0:1])
            nc.vector.tensor_copy(prime_t[:, 1:2], attc_t[:, 0:1])
            zero_t = singles.tile([P, 1], f32)
            nc.vector.memset(zero_t, 0.0)

            # ---- attention (collapsed), data-parallel over B -------------
            for bt in range(NBT):
                xv = x_t[:, bt, :]                       # [128, F]
                # alpha[p, h, i] = ah[h] * x[p, i] + ch[h]
                alpha_t = att.tile([P, NH, F], f32, tag="alpha")
                nc.vector.tensor_tensor(
                    out=alpha_t,
                    in0=xv.unsqueeze(1).broadcast_to([P, NH, F]),
                    in1=attc_t[:, 0:NH].unsqueeze(2).broadcast_to([P, NH, F]),
                    op=ALU.mult,
                )
                nc.vector.tensor_tensor(
                    out=alpha_t,
                    in0=alpha_t,
                    in1=attc_t[:, NH:2 * NH].unsqueeze(2).broadcast_to([P, NH, F]),
                    op=ALU.add,
                )
                for q in range(NH):
                    # S[p, i, j] = alpha[p, q, i] * x[p, j]
                    S_t = att1.tile([P, F, F], f32, tag="S")
                    nc.vector.tensor_tensor(
                        out=S_t,
                        in0=alpha_t[:, q, :].unsqueeze(2).broadcast_to([P, F, F]),
                        in1=xv.unsqueeze(1).broadcast_to([P, F, F]),
                        op=ALU.mult,
                    )
                    # E = exp(S)
                    E_t = att.tile([P, F, F], f32, tag="E")
                    nc.scalar.activation(E_t, S_t, AF.Exp)
                    # D = sum_j E ; numer = sum_j E*x_j
                    D_t = attsm.tile([P, F], f32, tag="D")
                    nc.vector.tensor_reduce(
                        out=D_t, in_=E_t, axis=AXL.X, op=ALU.add)
                    T_t = att1.tile([P, F, F], f32, tag="T")
                    nc.vector.tensor_tensor(
                        out=T_t, in0=E_t,
                        in1=xv.unsqueeze(1).broadcast_to([P, F, F]),
                        op=ALU.mult,
                    )
                    N_t = attsm.tile([P, F], f32, tag="N")
                    nc.vector.tensor_reduce(
                        out=N_t, in_=T_t, axis=AXL.X, op=ALU.add)
                    rD_t = attsm.tile([P, F], f32, tag="rD")
                    nc.vector.reciprocal(rD_t, D_t)
                    # attn = E * rD  (scalar_tensor_tensor: its instruction
                    # struct has more sync-wait slots than TensorTensor)
                    A_t = att.tile([P, F, F], f32, tag="A")
                    nc.vector.scalar_tensor_tensor(
                        out=A_t, in0=E_t, scalar=1.0,
                        in1=rD_t.unsqueeze(2).broadcast_to([P, F, F]),
                        op0=ALU.mult, op1=ALU.mult,
                    )
                    nc.sync.dma_start(
                        out=attn_d[bt * BT:(bt + 1) * BT, q, :, :], in_=A_t)
                    # s = numer * rD  (bf16 out)
                    nc.vector.tensor_tensor(
                        out=s_t[:, bt, q].rearrange("p d f -> p (d f)"),
                        in0=N_t, in1=rD_t, op=ALU.mult)
                    # scatter s block into the AllToAll input: dest block d
                    # gets features d*FL..  (b fast in dram)
                    for d in range(NCORES):
                        nc.sync.dma_start(
                            out=s2a_in[d, q, :, :]
                            .rearrange("f b -> b f")[bt * BT:(bt + 1) * BT],
                            in_=s_t[:, bt, q, d],
                        )

            # ---- AllToAll: redistribute s batch-shard -> feature-shard ---
            nc.gpsimd.collective_compute(
                "AllToAll",
                mybir.AluOpType.bypass,
                replica_groups=[list(range(NCORES))],
                ins=[s2a_in[:, :, :, :]],
                outs=[s2a_out[:, :, :, :]],
            )

            # ---- per-feature: M-proj -> enc/dec stacks -> out-proj -------
            for fl in range(FL):
                # sT_f[nh, (src, b)] = s for this feature, all B tokens
                sT_f = stpool.tile([NH, NCORES, BL], bf16, tag="sT")
                nc.sync.dma_start(
                    out=sT_f,
                    in_=s2a_out[:, :, fl, :].rearrange("r h b -> h r b"))

                # M-proj: zT0[o, t] = Mproj.T @ sT_f + const_a
                zT = zpool.tile([P, 2, TOK], bf16, tag="z")
                sT_flat = sT_f.rearrange("h r b -> h (r b)")
                for hc in range(2):
                    for n0 in range(0, TOK, RCH):
                        g_ps = ps_g.tile([P, RCH], f32, tag="g")
                        for c0 in range(0, RCH, NCH):
                            nc.tensor.matmul(
                                g_ps[:, c0:c0 + NCH],
                                mproj_t[:, hc * P:(hc + 1) * P],
                                sT_flat[:, n0 + c0:n0 + c0 + NCH],
                                start=True, stop=True,
                            )
                        nc.scalar.activation(
                            zT[:, hc, n0:n0 + RCH], g_ps, AF.Identity,
                            bias=ca_t[:, hc:hc + 1])

                # 4 encoder + 4 decoder layers
                for l in range(2 * NL):
                    w_d = wenc_d if l < NL else wdec_d
                    b_d = benc_d if l < NL else bdec_d
                    lj = l if l < NL else l - NL
                    w_t = wpool.tile([P, 2, 2, P], bf16, tag="w")
                    nc.sync.dma_start(
                        out=w_t,
                        in_=w_d[fl, lj].rearrange("(c p) (m q) -> p c m q",
                                                  p=P, q=P))
                    b_t = wpool.tile([P, 2], f32, tag="b")
                    nc.sync.dma_start(
                        out=b_t, in_=b_d[fl, lj].rearrange("(c p) -> p c", p=P))
                    zT_next = zpool.tile([P, 2, TOK], bf16, tag="z")
                    for m in range(2):
                        for n0 in range(0, TOK, RCH):
                            g_ps = ps_g.tile([P, RCH], f32, tag="g")
                            for c0 in range(0, RCH, NCH):
                                nc.tensor.matmul(
                                    g_ps[:, c0:c0 + NCH],
                                    w_t[:, 0, m, :],
                                    zT[:, 0, n0 + c0:n0 + c0 + NCH],
                                    start=True, stop=False)
                                nc.tensor.matmul(
                                    g_ps[:, c0:c0 + NCH],
                                    w_t[:, 1, m, :],
                                    zT[:, 1, n0 + c0:n0 + c0 + NCH],
                                    start=False, stop=True)
                            # relu+bias, alternating engines (ACT / DVE)
                            dst = zT_next[:, m, n0:n0 + RCH]
                            if (l + m) % 2 == 0:
                                nc.scalar.activation(
                                    dst, g_ps, AF.Relu, bias=b_t[:, m:m + 1])
                            else:
                                nc.vector.scalar_tensor_tensor(
                                    out=dst, in0=g_ps, scalar=b_t[:, m:m + 1],
                                    in1=zero_t.broadcast_to([P, RCH]),
                                    op0=ALU.add, op1=ALU.max, )
                    zT = zT_next

                # out-proj + sigmoid: sig[o=64, t]
                sig_t = opool.tile([OUT, TOK], bf16, tag="sig")
                for n0 in range(0, TOK, NCH):
                    o_ps = ps_o.tile([OUT, NCH], f32, tag="o")
                    nc.tensor.matmul(
                        o_ps, wout_t[:, 0, :], zT[:, 0, n0:n0 + NCH],
                        start=True, stop=False)
                    nc.tensor.matmul(
                        o_ps, wout_t[:, 1, :], zT[:, 1, n0:n0 + NCH],
                        start=False, stop=True)
                    nc.scalar.activation(sig_t[:, n0:n0 + NCH], o_ps,
                                         AF.Sigmoid, bias=bout_t[:, 0:1])
                # transpose [64, 128] -> [128, 64] per token tile, DMA out
                for tt in range(TOK // P):
                    t_ps = ps_t.tile([P, OUT], bf16, tag="t")
                    nc.tensor.transpose(
                        t_ps, sig_t[:, tt * P:(tt + 1) * P], ident)
                    oc_t = ocp.tile([P, OUT], f32, tag="oc")
                    nc.vector.tensor_copy(oc_t, t_ps)
                    nc.sync.dma_start(
                        out=pc_d[tt * P:(tt + 1) * P, fl, :], in_=oc_t)

    nc.compile()
    return nc


def kernel(**inputs):
    import sys
    for p in ("/opt/trn_rl_repo", "/opt/pypackages"):
        if p not in sys.path:
            sys.path.insert(0, p)
    from concourse.bass_utils import run_bass_kernel_spmd
    import ml_dtypes

    bf = ml_dtypes.bfloat16
    ah, ch, Mproj, const_a = _host_precompute(inputs)
    attc = np.concatenate([ah, ch]).astype(np.float32)

    x = np.ascontiguousarray(np.asarray(inputs["x"], dtype=np.float32))
    wenc = np.asarray(inputs["Wenc"], np.float32).astype(bf)
    wdec = np.asarray(inputs["Wdec"], np.float32).astype(bf)
    benc = np.asarray(inputs["benc"], np.float32)
    bdec = np.asarray(inputs["bdec"], np.float32)
    shared = {
        "attc": attc,
        "mproj": np.ascontiguousarray(Mproj.astype(bf)),
        "consta": np.ascontiguousarray(const_a),
        "wout": np.ascontiguousarray(
            np.asarray(inputs["Wout"], np.float32).astype(bf)),
        "bout": np.ascontiguousarray(np.asarray(inputs["bout"], np.float32)),
    }
    in_maps = [
        {
            "xs": np.ascontiguousarray(x[i * BL:(i + 1) * BL]),
            "wenc": np.ascontiguousarray(wenc[i * FL:(i + 1) * FL]),
            "benc": np.ascontiguousarray(benc[i * FL:(i + 1) * FL]),
            "wdec": np.ascontiguousarray(wdec[i * FL:(i + 1) * FL]),
            "bdec": np.ascontiguousarray(bdec[i * FL:(i + 1) * FL]),
            **shared,
        }
        for i in range(NCORES)
    ]

    nc = _build_graph()
    res = run_bass_kernel_spmd(nc, in_maps, core_ids=list(range(NCORES)))
    # pc: feature-sharded [TOK, FL, OUT] per core -> concat on feature axis
    pc = np.concatenate([res.results[i]["out_pc"] for i in range(NCORES)],
                        axis=1)
    # attn: batch-sharded
    attn = np.concatenate([res.results[i]["out_attn"] for i in range(NCORES)],
                          axis=0)
    return pc.astype(np.float32), attn.astype(np.float32)


# revision 20
# speedup vs baseline: 1.2117x; 1.2117x over previous
"""Trainium2 Bass kernel for nn_AttentiveAutoEncoder.

Key structure: the input embedding is Linear(1,E), so the token embedding
h[b,f,:] = x[b,f] * W_emb[0,:] is rank-1.  All of q/k/v and the MHA in_proj
outputs are therefore affine in the scalar x[b,f]:

    q2[b,f,:] = x[b,f]*u_q + c_q       (u_q, c_q host-precomputed [H])

so per head the attention scores collapse to

    scores[b,h,i,j] = (a_h x_i + c_h) x_j  +  (terms constant in j)

and the j-constant terms drop out of the softmax.  The context vector
collapses to ctx[b,i,head] = s[b,h,i] * u_v[head] + c_v with
s[b,h,i] = sum_j attn[b,h,i,j] x[b,j], so attention + out-proj becomes a
[*,NH] @ [NH,H] matmul.  Only the per-feature grouped MLP stacks remain as
real GEMM work.

Sharding: attention (and the attn output) is data-parallel over batch B.
The grouped MLP stacks are expert-parallel: each core owns F/8 = 8
features for ALL B tokens, so each 128x128 weight load is amortized over
N=2048 moving columns (4 matmuls of N=512) and the big [F,NL,H,H] weight
tensors are sharded 8x instead of replicated.  The only cross-core
exchange is an AllToAll of the tiny collapsed-attention tensor
s[B,NH,F] (~1 MB chip-wide).  Activations stay transposed ([H, tokens])
through the stack so no per-layer transposes are needed.
"""

import numpy as np

B, F, E, H, NH, OUT, NL = 2048, 64, 8, 256, 4, 64, 4
HD = H // NH
NCORES = 8
BL = B // NCORES          # 256 batch rows per core (attention shard)
BT = 128                  # batch tile (partition dim)
NBT = BL // BT            # 2 batch tiles per core
P = 128
FL = F // NCORES          # 8 features per core (grouped-GEMM shard)
TOK = B                   # tokens per feature in the grouped stacks
NCH = 512                 # matmul moving-dim chunk
RCH = 1024                # relu chunk (2 PSUM banks)


def _host_precompute(inp):
    """Collapse the attention block into a handful of small tensors."""
    f64 = lambda k: np.asarray(inp[k], dtype=np.float64)
    W_emb, Wq, bq = f64("W_emb"), f64("Wq"), f64("bq")
    Wk, bk, Wv, bv = f64("Wk"), f64("bk"), f64("Wv"), f64("bv")
    Win, bin_, Wo, bo = f64("Win"), f64("bin_"), f64("Wo"), f64("bo")
    Wq2, Wk2, Wv2 = np.split(Win, 3, axis=0)
    bq2, bk2, bv2 = np.split(bin_, 3)
    e = W_emb[0]
    uq = (e @ Wq) @ Wq2.T
    cq = bq @ Wq2.T + bq2
    uk = (e @ Wk) @ Wk2.T
    uv = (e @ Wv) @ Wv2.T
    cv = bv @ Wv2.T + bv2
    sc = 1.0 / np.sqrt(HD)
    ah = np.array([uq[h * HD:(h + 1) * HD] @ uk[h * HD:(h + 1) * HD]
                   for h in range(NH)]) * sc
    ch = np.array([cq[h * HD:(h + 1) * HD] @ uk[h * HD:(h + 1) * HD]
                   for h in range(NH)]) * sc
    # a[b,i,:] = sum_h s[b,h,i] * Mproj[h,:] + const_a
    Mproj = np.stack([uv[h * HD:(h + 1) * HD] @ Wo[:, h * HD:(h + 1) * HD].T
                      for h in range(NH)])          # [NH, H]
    const_a = cv @ Wo.T + bo                        # [H]
    return (ah.astype(np.float32), ch.astype(np.float32),
            Mproj.astype(np.float32), const_a.astype(np.float32))


def _build_graph():
    import concourse.bass as bass
    import concourse.mybir as mybir
    import concourse.tile as tile
    from concourse import bacc
    from concourse.masks import make_identity

    f32 = mybir.dt.float32
    bf16 = mybir.dt.bfloat16
    AF = mybir.ActivationFunctionType
    ALU = mybir.AluOpType
    AXL = mybir.AxisListType

    nc = bacc.Bacc(None)

    x_d = nc.declare_dram_parameter("xs", [BL, F], f32, isOutput=False)
    attc_d = nc.declare_dram_parameter("attc", [2 * NH], f32, isOutput=False)
    mproj_d = nc.declare_dram_parameter("mproj", [NH, H], bf16, isOutput=False)
    ca_d = nc.declare_dram_parameter("consta", [H], f32, isOutput=False)
    # weights+bias pre-packed on host: [fl, l, p, (c,m,q)+c] (bf16, dense)
    wb_d = nc.declare_dram_parameter("wb", [FL, 2 * NL, P, 516], bf16,
                                     isOutput=False)
    wout_d = nc.declare_dram_parameter("wout", [H, OUT], bf16, isOutput=False)
    bout_d = nc.declare_dram_parameter("bout", [OUT], f32, isOutput=False)
    pc_d = nc.declare_dram_parameter("out_pc", [TOK, FL, OUT], f32,
                                     isOutput=True)
    attn_d = nc.declare_dram_parameter("out_attn", [BL, NH, F, F], f32,
                                       isOutput=True)
    # AllToAll bounce buffers for s: block d (for dest core d) holds
    # s[nh, features d*FL..(d+1)*FL, local b].  After the collective,
    # s2a_out[src, nh, fl, b] = s of THIS core's features for src's batch.
    s2a_in = nc.dram_tensor("s_a2a_in", [NCORES, NH, FL, BL], bf16)
    s2a_out = nc.dram_tensor("s_a2a_out", [NCORES, NH, FL, BL], bf16)

    with tile.TileContext(nc) as tc:
        with (
            tc.tile_pool(name="singles", bufs=1) as singles,
            tc.tile_pool(name="att", bufs=2) as att,
            tc.tile_pool(name="att1", bufs=1) as att1,
            tc.tile_pool(name="attsm", bufs=2) as attsm,
            tc.tile_pool(name="stp", bufs=2) as stpool,
            tc.tile_pool(name="wpool", bufs=8) as wpool,
            tc.tile_pool(name="zpool", bufs=2) as zpool,
            tc.tile_pool(name="opool", bufs=2) as opool,
            tc.tile_pool(name="ocp", bufs=4) as ocp,
            tc.tile_pool(name="ps_g", bufs=2, space="PSUM") as ps_g,
            tc.tile_pool(name="ps_o", bufs=2, space="PSUM") as ps_o,
            tc.tile_pool(name="ps_t", bufs=2, space="PSUM") as ps_t,
        ):
            # ---- one-time setup ------------------------------------------
            attc_t = singles.tile([P, 2 * NH], f32)
            nc.sync.dma_start(out=attc_t,
                              in_=attc_d[:].partition_broadcast(P))
            x_t = singles.tile([P, NBT, F], f32)
            nc.sync.dma_start(
                out=x_t, in_=x_d[:, :].rearrange("(t p) f -> p t f", p=P))
            mproj_t = singles.tile([NH, H], bf16)
            nc.sync.dma_start(out=mproj_t, in_=mproj_d[:, :])
            ca_t = singles.tile([P, 2], f32)
            nc.sync.dma_start(out=ca_t,
                              in_=ca_d[:].rearrange("(c p) -> p c", p=P))
            wout_t = singles.tile([P, 2, OUT], bf16)
            nc.sync.dma_start(
                out=wout_t, in_=wout_d[:, :].rearrange("(c p) o -> p c o", p=P))
            bout_t = singles.tile([OUT, 1], f32)
            nc.sync.dma_start(out=bout_t,
                              in_=bout_d[:].rearrange("(o u) -> o u", u=1))
            ident = singles.tile([OUT, OUT], bf16)
            make_identity(nc, ident)

            # s for all heads, both b tiles: [128, NBT, NH, dest, FL] (bf16)
            s_t = singles.tile([P, NBT, NH, NCORES, FL], bf16)

            # prime DVE's vector clock on the setup DMAs so the 1-wait-slot
            # TensorTensor instructions below never need >1 sync wait
            prime_t = singles.tile([P, 2], f32)
            nc.vector.tensor_copy(prime_t[:, 0:1], x_t[:, 0, 0:1])
            nc.vector.tensor_copy(prime_t[:, 1:2], attc_t[:, 0:1])
            zero_t = singles.tile([P, 1], f32)
            nc.vector.memset(zero_t, 0.0)

            # ---- attention (collapsed), data-parallel over B -------------
            for bt in range(NBT):
                xv = x_t[:, bt, :]                       # [128, F]
                # alpha[p, h, i] = ah[h] * x[p, i] + ch[h]
                alpha_t = att.tile([P, NH, F], f32, tag="alpha")
                nc.vector.tensor_tensor(
                    out=alpha_t,
                    in0=xv.unsqueeze(1).broadcast_to([P, NH, F]),
                    in1=attc_t[:, 0:NH].unsqueeze(2).broadcast_to([P, NH, F]),
                    op=ALU.mult,
                )
                nc.vector.tensor_tensor(
                    out=alpha_t,
                    in0=alpha_t,
                    in1=attc_t[:, NH:2 * NH].unsqueeze(2).broadcast_to([P, NH, F]),
                    op=ALU.add,
                )
                for q in range(NH):
                    # S[p, i, j] = alpha[p, q, i] * x[p, j]
                    S_t = att1.tile([P, F, F], f32, tag="S")
                    nc.vector.tensor_tensor(
                        out=S_t,
                        in0=alpha_t[:, q, :].unsqueeze(2).broadcast_to([P, F, F]),
                        in1=xv.unsqueeze(1).broadcast_to([P, F, F]),
                        op=ALU.mult,
                    )
                    # E = exp(S)
                    E_t = att.tile([P, F, F], f32, tag="E")
                    nc.scalar.activation(E_t, S_t, AF.Exp)
                    # D = sum_j E ; numer = sum_j E*x_j
                    D_t = attsm.tile([P, F], f32, tag="D")
                    nc.vector.tensor_reduce(
                        out=D_t, in_=E_t, axis=AXL.X, op=ALU.add)
                    T_t = att1.tile([P, F, F], f32, tag="T")
                    nc.vector.tensor_tensor(
                        out=T_t, in0=E_t,
                        in1=xv.unsqueeze(1).broadcast_to([P, F, F]),
                        op=ALU.mult,
                    )
                    N_t = attsm.tile([P, F], f32, tag="N")
                    nc.vector.tensor_reduce(
                        out=N_t, in_=T_t, axis=AXL.X, op=ALU.add)
                    rD_t = attsm.tile([P, F], f32, tag="rD")
                    nc.vector.reciprocal(rD_t, D_t)
                    # attn = E * rD  (scalar_tensor_tensor: its instruction
                    # struct has more sync-wait slots than TensorTensor)
                    A_t = att.tile([P, F, F], f32, tag="A")
                    nc.vector.scalar_tensor_tensor(
                        out=A_t, in0=E_t, scalar=1.0,
                        in1=rD_t.unsqueeze(2).broadcast_to([P, F, F]),
                        op0=ALU.mult, op1=ALU.mult,
                    )
                    nc.sync.dma_start(
                        out=attn_d[bt * BT:(bt + 1) * BT, q, :, :], in_=A_t)
                    # s = numer * rD  (bf16 out)
                    nc.vector.tensor_tensor(
                        out=s_t[:, bt, q].rearrange("p d f -> p (d f)"),
                        in0=N_t, in1=rD_t, op=ALU.mult)
                    # scatter s block into the AllToAll input: dest block d
                    # gets features d*FL..  (b fast in dram)
                    for d in range(NCORES):
                        nc.gpsimd.dma_start(
                            out=s2a_in[d, q, :, :]
                            .rearrange("f b -> b f")[bt * BT:(bt + 1) * BT],
                            in_=s_t[:, bt, q, d],
                        )

            # ---- AllToAll: redistribute s batch-shard -> feature-shard ---
            nc.gpsimd.collective_compute(
                "AllToAll",
                mybir.AluOpType.bypass,
                replica_groups=[list(range(NCORES))],
                ins=[s2a_in[:, :, :, :]],
                outs=[s2a_out[:, :, :, :]],
            )

            # ---- per-feature: M-proj -> enc/dec stacks -> out-proj -------
            for fl in range(FL):
                # sT_f[nh, (src, b)] = s for this feature, all B tokens
                sT_f = stpool.tile([NH, NCORES, BL], bf16, tag="sT")
                nc.sync.dma_start(
                    out=sT_f,
                    in_=s2a_out[:, :, fl, :].rearrange("r h b -> h r b"))

                # M-proj: zT0[o, t] = Mproj.T @ sT_f + const_a
                zT = zpool.tile([P, 2, TOK], bf16, tag="z")
                sT_flat = sT_f.rearrange("h r b -> h (r b)")
                for hc in range(2):
                    for n0 in range(0, TOK, RCH):
                        g_ps = ps_g.tile([P, RCH], f32, tag="g")
                        for c0 in range(0, RCH, NCH):
                            nc.tensor.matmul(
                                g_ps[:, c0:c0 + NCH],
                                mproj_t[:, hc * P:(hc + 1) * P],
                                sT_flat[:, n0 + c0:n0 + c0 + NCH],
                                start=True, stop=True,
                            )
                        nc.scalar.activation(
                            zT[:, hc, n0:n0 + RCH], g_ps, AF.Identity,
                            bias=ca_t[:, hc:hc + 1])

                # 4 encoder + 4 decoder layers
                for l in range(2 * NL):
                    wb_t = wpool.tile([P, 516], bf16, tag="w")
                    nc.sync.dma_start(out=wb_t, in_=wb_d[fl, l])
                    w_t = wb_t[:, 0:512].rearrange("p (c m q) -> p c m q",
                                                   c=2, m=2)
                    b_t = wb_t[:, 512:516].rearrange("p (c u) -> p c u", c=2)
                    zT_next = zpool.tile([P, 2, TOK], bf16, tag="z")
                    for m in range(2):
                        for n0 in range(0, TOK, RCH):
                            g_ps = ps_g.tile([P, RCH], f32, tag="g")
                            for c0 in range(0, RCH, NCH):
                                nc.tensor.matmul(
                                    g_ps[:, c0:c0 + NCH],
                                    w_t[:, 0, m, :],
                                    zT[:, 0, n0 + c0:n0 + c0 + NCH],
                                    start=True, stop=False)
                                nc.tensor.matmul(
                                    g_ps[:, c0:c0 + NCH],
                                    w_t[:, 1, m, :],
                                    zT[:, 1, n0 + c0:n0 + c0 + NCH],
                                    start=False, stop=True)
                            # relu+bias, alternating engines (ACT / DVE)
                            dst = zT_next[:, m, n0:n0 + RCH]
                            if (l + m) % 2 == 0:
                                nc.scalar.activation(
                                    dst, g_ps, AF.Relu,
                                    bias=b_t[:, m, 0:1])
                            else:
                                nc.vector.scalar_tensor_tensor(
                                    out=dst, in0=g_ps,
                                    scalar=b_t[:, m, 0:1],
                                    in1=zero_t.broadcast_to([P, RCH]),
                                    op0=ALU.add, op1=ALU.max, )
                    zT = zT_next

                # out-proj + sigmoid: sig[o=64, t]
                sig_t = opool.tile([OUT, TOK], bf16, tag="sig")
                for n0 in range(0, TOK, NCH):
                    o_ps = ps_o.tile([OUT, NCH], f32, tag="o")
                    nc.tensor.matmul(
                        o_ps, wout_t[:, 0, :], zT[:, 0, n0:n0 + NCH],
                        start=True, stop=False)
                    nc.tensor.matmul(
                        o_ps, wout_t[:, 1, :], zT[:, 1, n0:n0 + NCH],
                        start=False, stop=True)
                    nc.scalar.activation(sig_t[:, n0:n0 + NCH], o_ps,
                                         AF.Sigmoid, bias=bout_t[:, 0:1])
                # transpose [64, 128] -> [128, 64] per token tile, DMA out
                for tt in range(TOK // P):
                    t_ps = ps_t.tile([P, OUT], bf16, tag="t")
                    nc.tensor.transpose(
                        t_ps, sig_t[:, tt * P:(tt + 1) * P], ident)
                    oc_t = ocp.tile([P, OUT], f32, tag="oc")
                    nc.vector.tensor_copy(oc_t, t_ps)
                    nc.gpsimd.dma_start(
                        out=pc_d[tt * P:(tt + 1) * P, fl, :], in_=oc_t)

    nc.compile()
    return nc


def kernel(**inputs):
    import sys
    for p in ("/opt/trn_rl_repo", "/opt/pypackages"):
        if p not in sys.path:
            sys.path.insert(0, p)
    from concourse.bass_utils import run_bass_kernel_spmd
    import ml_dtypes

    bf = ml_dtypes.bfloat16
    ah, ch, Mproj, const_a = _host_precompute(inputs)
    attc = np.concatenate([ah, ch]).astype(np.float32)

    x = np.ascontiguousarray(np.asarray(inputs["x"], dtype=np.float32))
    # pack weights+biases: wb[f, l, p, 0:512] = W[l][c*128+p, m*128+q] at
    # j=(c*2+m)*128+q ; wb[f, l, p, 512+c*2] = b[l][c*128+p]
    wall = np.concatenate([np.asarray(inputs["Wenc"], np.float32),
                           np.asarray(inputs["Wdec"], np.float32)],
                          axis=1)                       # [F, 2NL, H, H]
    ball = np.concatenate([np.asarray(inputs["benc"], np.float32),
                           np.asarray(inputs["bdec"], np.float32)],
                          axis=1)                       # [F, 2NL, H]
    wr = wall.reshape(F, 2 * NL, 2, P, 2, P)            # [f,l,c,p,m,q]
    wr = wr.transpose(0, 1, 3, 2, 4, 5).reshape(F, 2 * NL, P, 512)
    br = ball.reshape(F, 2 * NL, 2, P).transpose(0, 1, 3, 2)  # [f,l,p,c]
    wb = np.zeros((F, 2 * NL, P, 516), np.float32)
    wb[..., 0:512] = wr
    wb[..., 512] = br[..., 0]
    wb[..., 514] = br[..., 1]
    wb = wb.astype(bf)
    shared = {
        "attc": attc,
        "mproj": np.ascontiguousarray(Mproj.astype(bf)),
        "consta": np.ascontiguousarray(const_a),
        "wout": np.ascontiguousarray(
            np.asarray(inputs["Wout"], np.float32).astype(bf)),
        "bout": np.ascontiguousarray(np.asarray(inputs["bout"], np.float32)),
    }
    in_maps = [
        {
            "xs": np.ascontiguousarray(x[i * BL:(i + 1) * BL]),
            "wb": np.ascontiguousarray(wb[i * FL:(i + 1) * FL]),
            **shared,
        }
        for i in range(NCORES)
    ]

    nc = _build_graph()
    res = run_bass_kernel_spmd(nc, in_maps, core_ids=list(range(NCORES)))
    # pc: feature-sharded [TOK, FL, OUT] per core -> concat on feature axis
    pc = np.concatenate([res.results[i]["out_pc"] for i in range(NCORES)],
                        axis=1)
    # attn: batch-sharded
    attn = np.concatenate([res.results[i]["out_attn"] for i in range(NCORES)],
                          axis=0)
    return pc.astype(np.float32), attn.astype(np.float32)
